# revision 39
# baseline (speedup 1.0000x reference)
"""MoE FFN (grouped top-1 routing, SwiGLU experts) on 8 Trainium2 NeuronCores.

Strategy (expert-parallel, quarter-expert load balancing):
  - Host computes the routers (sigmoid macro top-1 group of 4; both experts of
    the selected group active with normalized sigmoid weights).
  - Each expert's FFN dim F=2048 is split into four quarter-experts (F_q=512).
    Every core processes exactly ONE quarter-expert from EACH of the 4 groups
    (bijection (j, q) = (core%2, core//2) per group), so per-core work is the
    perfectly balanced sum over all group sizes — immune to routing imbalance.
  - Per-token router weight w2 is folded into the OUTPUT column scale
    (y = w2 ⊙ down_q^T(silu(gate_q^T x) * (up_q^T x))), so tokens are shipped
    once (bf16) and no weighted copy is needed. Host sums the 8 partials
    (2 experts x 4 quarters) per token.
  - DMA is a single strict-priority queue (tokens + weights task-by-task),
    outputs stream out on the scalar queue, and the PE is pre-warmed with
    dummy matmuls so the HAM clock gate reaches 2.4 GHz early.
"""

import math

import ml_dtypes
import numpy as np

import concourse.bass as bass  # noqa: F401  (bass types via bacc)
import concourse.mybir as mybir
import concourse.tile as tile
from concourse import bacc
from concourse.bass_utils import run_bass_kernel_spmd

P = 128
D_MODEL = 1024
FFN_DIM = 2048
N_QUART = 4
F_Q = FFN_DIM // N_QUART  # 512
FO_Q = F_Q // P  # 4 f-tiles per quarter-expert
DO = D_MODEL // P  # 8 k-tiles over D
NUM_EXPERTS = 8
NUM_GROUPS = 4
EPS = 1e-9

F32 = mybir.dt.float32
BF16 = mybir.dt.bfloat16

N_CORES = 8
N_WARM = 44  # PE warmup matmuls (HAM un-throttle)

_BUILD_CACHE: dict[tuple, object] = {}
LAST_RESULTS = None  # stashed BassKernelResults for test harnesses


def _task_phase1(nc, gt3, ut3, xts3, hs, pools, chunk, nch, tag, warm_fn=None):
    """SwiGLU up to h = silu(gate^T x) * (up^T x) for one quarter-expert."""
    sp, pgp, pup = pools
    for cc in range(nch):
        cs = slice(cc * chunk, (cc + 1) * chunk)
        for fo in range(FO_Q):
            fill = warm_fn if (warm_fn and cc == 0 and fo == 0) else None
            psg = pgp.tile([P, 512], F32, tag="psg", name=f"psg{tag}_{cc}_{fo}")[:, :chunk]
            psu = pup.tile([P, 512], F32, tag="psu", name=f"psu{tag}_{cc}_{fo}")[:, :chunk]
            for do in range(DO):
                nc.tensor.matmul(
                    psg[:],
                    gt3[:, fo * DO + do, :],
                    xts3[:, cc * DO + do, :],
                    start=(do == 0),
                    stop=(do == DO - 1),
                )
            if fill:
                fill(10)
            for do in range(DO):
                nc.tensor.matmul(
                    psu[:],
                    ut3[:, fo * DO + do, :],
                    xts3[:, cc * DO + do, :],
                    start=(do == 0),
                    stop=(do == DO - 1),
                )
            if fill:
                fill(8)
            sg = sp.tile([P, 512], F32, tag="sg", name=f"sg{tag}_{cc}_{fo}")[:, :chunk]
            nc.scalar.activation(sg[:], psg[:], mybir.ActivationFunctionType.Silu)
            nc.vector.tensor_mul(out=hs[:, fo, cs], in0=sg[:], in1=psu[:])


def _task_phase2(nc, dt3, hs, w2b, yt3, pools, chunk, nch, tag, oq):
    """y^T = w2 ⊙ (down_q^T @ h): router weight folded into the output copy."""
    yp, pdp = pools
    for cc in range(nch):
        cs = slice(cc * chunk, (cc + 1) * chunk)
        yo = yp.tile([P, DO, 512], BF16, tag="yo", name=f"yo{tag}_{cc}")[:, :, :chunk]
        for do in range(DO):
            psy = pdp.tile([P, 512], F32, tag="psy", name=f"psy{tag}_{cc}_{do}")[:, :chunk]
            for fo in range(FO_Q):
                nc.tensor.matmul(
                    psy[:],
                    dt3[:, do * FO_Q + fo, :],
                    hs[:, fo, cs],
                    start=(fo == 0),
                    stop=(fo == FO_Q - 1),
                )
            nc.vector.tensor_mul(out=yo[:, do, :], in0=psy[:], in1=w2b[:, cs])
            if do % 2 == 1:
                oq.dma_start(
                    yt3[:, cc * DO + do - 1 : cc * DO + do + 1, :],
                    yo[:, do - 1 : do + 1, :],
                )


def _build(shapes: tuple):
    """Bass/Tile program: four quarter-expert SwiGLU tasks per core."""
    nc = bacc.Bacc(
        "TRN2",
        target_bir_lowering=False,
        debug=False,
        enable_asserts=False,
        num_devices=N_CORES,
    )
    NT = len(shapes)
    xt, w2, gw, uw, dw, yt = [], [], [], [], [], []
    for t, (C, nch) in enumerate(shapes):
        xt.append(nc.dram_tensor(f"xt{t}", [P, DO * C], BF16, kind="ExternalInput").ap())
        w2.append(nc.dram_tensor(f"w2{t}", [1, C], BF16, kind="ExternalInput").ap())
        gw.append(nc.dram_tensor(f"gw{t}", [P, FO_Q * DO * P], BF16, kind="ExternalInput").ap())
        uw.append(nc.dram_tensor(f"uw{t}", [P, FO_Q * DO * P], BF16, kind="ExternalInput").ap())
        dw.append(nc.dram_tensor(f"dw{t}", [P, DO * FO_Q * P], BF16, kind="ExternalInput").ap())
        yt.append(nc.dram_tensor(f"yt{t}", [P, DO * C], BF16, kind="ExternalOutput").ap())
    g3 = [g.rearrange("p (x f) -> p x f", f=P) for g in gw]
    u3 = [u.rearrange("p (x f) -> p x f", f=P) for u in uw]
    d3 = [d.rearrange("p (x f) -> p x f", f=P) for d in dw]
    xt3 = [x.rearrange("p (x c) -> p x c", c=C // nch) for x, (C, nch) in zip(xt, shapes)]
    yt3 = [y.rearrange("p (x c) -> p x c", c=C // nch) for y, (C, nch) in zip(yt, shapes)]

    with tile.TileContext(nc) as tc:
        with (
            tc.tile_pool(name="big", bufs=1) as big,
            tc.tile_pool(name="sp", bufs=4) as sp,
            tc.tile_pool(name="yp", bufs=2) as yp,
            tc.tile_pool(name="pg", bufs=2, space="PSUM") as pgp,
            tc.tile_pool(name="pu", bufs=2, space="PSUM") as pup,
            tc.tile_pool(name="pd", bufs=3, space="PSUM") as pdp,
            tc.tile_pool(name="pw", bufs=1, space="PSUM") as pwp,
        ):
            # ── persistent tiles ────────────────────────────────────────
            wkt = big.tile([P, P], BF16, tag="wkt")
            xts, w2r, w2b, gts, uts, dts, hss = [], [], [], [], [], [], []
            for t, (C, nch) in enumerate(shapes):
                chunk = C // nch
                xts.append(big.tile([P, nch * DO, chunk], BF16, tag=f"xts{t}", name=f"xts{t}"))
                w2r.append(big.tile([1, C], BF16, tag=f"w2r{t}", name=f"w2r{t}"))
                w2b.append(big.tile([P, C], BF16, tag=f"w2b{t}", name=f"w2b{t}"))
                gts.append(big.tile([P, FO_Q * DO, P], BF16, tag=f"gts{t}", name=f"gts{t}"))
                uts.append(big.tile([P, FO_Q * DO, P], BF16, tag=f"uts{t}", name=f"uts{t}"))
                dts.append(big.tile([P, DO * FO_Q, P], BF16, tag=f"dts{t}", name=f"dts{t}"))
                hss.append(big.tile([P, FO_Q, C], BF16, tag=f"hss{t}", name=f"hss{t}"))

            # ── PE warmup: memset a tile (no DMA dep), run dummy matmuls
            # so the HAM clock gate reaches 2.4 GHz before real work ──────
            nc.vector.memset(wkt[:], 0.25)
            wps = pwp.tile([P, P], F32, tag="wps")

            def _warm(n):
                for _ in range(n):
                    nc.tensor.matmul(wps[:], wkt[:], wkt[:], start=True, stop=True)

            _warm(N_WARM)

            # ── DMA schedule: ONE input queue (gpsimd), strict priority:
            # task-by-task (tokens, gate/up slabs, down, w2 row) ─────────
            for t, (C, nch) in enumerate(shapes):
                chunk = C // nch
                if t == 0:
                    nc.gpsimd.dma_start(xts[t][:, 0:4], xt3[t][:, 0:4])
                    nc.gpsimd.dma_start(xts[t][:, 4:DO], xt3[t][:, 4:DO])
                    wplan = [(0, 1), (1, 2), (2, 4)]
                else:
                    nc.gpsimd.dma_start(xts[t][:, 0:DO], xt3[t][:, 0:DO])
                    wplan = [(0, 2), (2, 4)]
                for lo, hi in wplan:
                    nc.gpsimd.dma_start(gts[t][:, lo * DO : hi * DO], g3[t][:, lo * DO : hi * DO])
                    nc.gpsimd.dma_start(uts[t][:, lo * DO : hi * DO], u3[t][:, lo * DO : hi * DO])
                for cc in range(1, nch):
                    csl = slice(cc * DO, (cc + 1) * DO)
                    nc.gpsimd.dma_start(xts[t][:, csl], xt3[t][:, csl])
                nc.gpsimd.dma_start(dts[t][:], d3[t][:])
                nc.gpsimd.dma_start(w2r[t][:], w2[t])
                nc.gpsimd.partition_broadcast(w2b[t][:], w2r[t][:])

            # ── tasks: heaviest first; P1 then P2 per task ──────────────
            for t, (C, nch) in enumerate(shapes):
                chunk = C // nch
                _task_phase1(
                    nc, gts[t], uts[t], xts[t], hss[t], (sp, pgp, pup),
                    chunk, nch, f"t{t}", warm_fn=_warm if t == 0 else None,
                )
                _task_phase2(
                    nc, dts[t], hss[t], w2b[t], yt3[t], (yp, pdp), chunk, nch,
                    f"t{t}", nc.scalar if t % 2 == 0 else nc.sync,
                )
    nc.finalize()
    return nc


def _get_program(key):
    if key not in _BUILD_CACHE:
        _BUILD_CACHE[key] = _build(key)
    return _BUILD_CACHE[key]


def _sigmoid(z):
    return 1.0 / (1.0 + np.exp(-z))


def _route(xf32, macro_w, micro_w):
    """Host routers in float64. Returns group index per token and per-token
    weights for the 2 experts of the selected group (float32)."""
    xf = xf32.astype(np.float64)
    ms = _sigmoid(xf @ macro_w.astype(np.float64))  # [T, G]
    g_sel = np.argmax(ms, axis=1)
    T = xf.shape[0]
    mval = ms[np.arange(T), g_sel]
    mv = mval / (mval + EPS)

    w2 = np.zeros((T, 2), np.float64)
    for g in range(NUM_GROUPS):
        idx = np.nonzero(g_sel == g)[0]
        if idx.size == 0:
            continue
        s = _sigmoid(xf[idx] @ micro_w[g].astype(np.float64))  # [n, 2]
        denom = np.maximum(s[:, 0], s[:, 1]) + np.minimum(s[:, 0], s[:, 1]) + EPS
        w2[idx, 0] = mv[idx] * s[:, 0] / denom
        w2[idx, 1] = mv[idx] * s[:, 1] / denom
    return g_sel, w2.astype(np.float32)


def _pick_capacity(n: int):
    n = max(n, 16)
    nch = max(1, -(-n // 512))
    chunk = -(-n // nch)
    chunk = -(-chunk // 4) * 4
    return chunk * nch, nch


def _pack_w(w):
    """[1024, 512] f32 -> [128, 4096] bf16, layout [p, outer4, inner8, 128]."""
    return np.ascontiguousarray(
        w.reshape(8, P, N_QUART, P).transpose(1, 2, 0, 3).reshape(P, FO_Q * DO * P)
    ).astype(ml_dtypes.bfloat16)


def _pack_wd(w):
    """[512, 1024] f32 -> [128, 4096] bf16, layout [p, outer8, inner4, 128]."""
    return np.ascontiguousarray(
        w.reshape(N_QUART, P, 8, P).transpose(1, 2, 0, 3).reshape(P, DO * FO_Q * P)
    ).astype(ml_dtypes.bfloat16)


def _pack_x(xg, C, nch, chunk):
    """tokens [n, 1024] f32 -> [128, 8*C] bf16, layout [p, cc, do, c]."""
    n = xg.shape[0]
    z = np.zeros((D_MODEL, C), np.float32)
    if n:
        z[:, :n] = xg.T
    return np.ascontiguousarray(
        z.reshape(DO, P, nch, chunk).transpose(1, 2, 0, 3).reshape(P, DO * C)
    ).astype(ml_dtypes.bfloat16)


def _unpack_y(r, C, nch, chunk, n):
    """[128, 8*C] bf16 -> [n, 1024] f32 token-major partial output."""
    y = (
        np.asarray(r, dtype=np.float32)
        .reshape(P, nch, DO, chunk)
        .transpose(2, 0, 1, 3)
        .reshape(D_MODEL, C)
    )
    return y[:, :n].T


def kernel(x, macro_w, micro_w, gate_w, up_w, down_w):
    global LAST_RESULTS
    x = np.asarray(x)
    B, S, D = x.shape
    T = B * S
    xf = np.ascontiguousarray(x.reshape(T, D).astype(np.float32, copy=False))

    g_sel, w2 = _route(xf, np.asarray(macro_w), np.asarray(micro_w))
    idx_by_g = [np.nonzero(g_sel == g)[0] for g in range(NUM_GROUPS)]
    sizes = np.array([ix.size for ix in idx_by_g])

    # tasks: one quarter-expert per group per core, heaviest group first
    order = [int(g) for g in np.argsort(-sizes, kind="stable")]
    shapes = []
    for g in order:
        C, nch = _pick_capacity(int(sizes[g]))
        shapes.append((C, nch))
    shapes = tuple(shapes)
    nc = _get_program(shapes)

    gate_w = np.asarray(gate_w, np.float32)
    up_w = np.asarray(up_w, np.float32)
    down_w = np.asarray(down_w, np.float32)

    in_maps = []
    for c in range(N_CORES):
        j = c % 2  # local expert within each group
        q = c // 2  # F-quarter
        fsl = slice(q * F_Q, (q + 1) * F_Q)
        m = {}
        for t, g_of in enumerate(order):
            C, nch = shapes[t]
            chunk = C // nch
            e = 2 * g_of + j
            ix = idx_by_g[g_of]
            m[f"xt{t}"] = _pack_x(xf[ix], C, nch, chunk)
            w2row = np.zeros((1, C), np.float32)
            w2row[0, : ix.size] = w2[ix, j]
            m[f"w2{t}"] = w2row.astype(ml_dtypes.bfloat16)
            m[f"gw{t}"] = _pack_w(gate_w[e][:, fsl])
            m[f"uw{t}"] = _pack_w(up_w[e][:, fsl])
            m[f"dw{t}"] = _pack_wd(down_w[e][fsl, :])
        in_maps.append(m)

    res = run_bass_kernel_spmd(nc, in_maps, core_ids=list(range(N_CORES)))
    LAST_RESULTS = res

    y = np.zeros((T, D), np.float32)
    for c in range(N_CORES):
        for t, g_of in enumerate(order):
            C, nch = shapes[t]
            chunk = C // nch
            ix = idx_by_g[g_of]
            if ix.size:
                y[ix] += _unpack_y(res.results[c][f"yt{t}"], C, nch, chunk, ix.size)
    return y.reshape(B, S, D)


# revision 40
# speedup vs baseline: 1.0084x; 1.0084x over previous
"""MoE FFN (grouped top-1 routing, SwiGLU experts) on 8 Trainium2 NeuronCores.

Strategy (expert-parallel, quarter-expert load balancing):
  - Host computes the routers (sigmoid macro top-1 group of 4; both experts of
    the selected group active with normalized sigmoid weights).
  - Each expert's FFN dim F=2048 is split into four quarter-experts (F_q=512).
    Every core processes exactly ONE quarter-expert from EACH of the 4 groups
    (bijection (j, q) = (core%2, core//2) per group), so per-core work is the
    perfectly balanced sum over all group sizes — immune to routing imbalance.
  - Per-token router weight w2 is folded into the OUTPUT column scale
    (y = w2 ⊙ down_q^T(silu(gate_q^T x) * (up_q^T x))), so tokens are shipped
    once (bf16) and no weighted copy is needed. Host sums the 8 partials
    (2 experts x 4 quarters) per token.
  - DMA is a single strict-priority queue (tokens + weights task-by-task),
    outputs stream out on the scalar queue, and the PE is pre-warmed with
    dummy matmuls so the HAM clock gate reaches 2.4 GHz early.
"""

import math

import ml_dtypes
import numpy as np

import concourse.bass as bass  # noqa: F401  (bass types via bacc)
import concourse.mybir as mybir
import concourse.tile as tile
from concourse import bacc
from concourse.bass_utils import run_bass_kernel_spmd

P = 128
D_MODEL = 1024
FFN_DIM = 2048
N_QUART = 4
F_Q = FFN_DIM // N_QUART  # 512
FO_Q = F_Q // P  # 4 f-tiles per quarter-expert
DO = D_MODEL // P  # 8 k-tiles over D
NUM_EXPERTS = 8
NUM_GROUPS = 4
EPS = 1e-9

F32 = mybir.dt.float32
BF16 = mybir.dt.bfloat16

N_CORES = 8
N_WARM = 44  # PE warmup matmuls (HAM un-throttle)

_BUILD_CACHE: dict[tuple, object] = {}
LAST_RESULTS = None  # stashed BassKernelResults for test harnesses


def _task_phase1(nc, gt3, ut3, xts3, hs, pools, chunk, nch, tag, warm_fn=None):
    """SwiGLU up to h = silu(gate^T x) * (up^T x) for one quarter-expert."""
    sp, pgp, pup = pools
    for cc in range(nch):
        cs = slice(cc * chunk, (cc + 1) * chunk)
        for fo in range(FO_Q):
            fill = warm_fn if (warm_fn and cc == 0 and fo == 0) else None
            psg = pgp.tile([P, 512], F32, tag="psg", name=f"psg{tag}_{cc}_{fo}")[:, :chunk]
            psu = pup.tile([P, 512], F32, tag="psu", name=f"psu{tag}_{cc}_{fo}")[:, :chunk]
            for do in range(DO):
                nc.tensor.matmul(
                    psg[:],
                    gt3[:, fo * DO + do, :],
                    xts3[:, cc * DO + do, :],
                    start=(do == 0),
                    stop=(do == DO - 1),
                )
            if fill:
                fill(10)
            for do in range(DO):
                nc.tensor.matmul(
                    psu[:],
                    ut3[:, fo * DO + do, :],
                    xts3[:, cc * DO + do, :],
                    start=(do == 0),
                    stop=(do == DO - 1),
                )
            if fill:
                fill(8)
            sg = sp.tile([P, 512], F32, tag="sg", name=f"sg{tag}_{cc}_{fo}")[:, :chunk]
            nc.scalar.activation(sg[:], psg[:], mybir.ActivationFunctionType.Silu)
            nc.vector.tensor_mul(out=hs[:, fo, cs], in0=sg[:], in1=psu[:])


def _task_phase2(nc, dt3, hs, w2b, yt3, pools, chunk, nch, tag, oq):
    """y^T = w2 ⊙ (down_q^T @ h): router weight folded into the output copy."""
    yp, pdp = pools
    for cc in range(nch):
        cs = slice(cc * chunk, (cc + 1) * chunk)
        yo = yp.tile([P, DO, 512], BF16, tag="yo", name=f"yo{tag}_{cc}")[:, :, :chunk]
        for do in range(DO):
            psy = pdp.tile([P, 512], F32, tag="psy", name=f"psy{tag}_{cc}_{do}")[:, :chunk]
            for fo in range(FO_Q):
                nc.tensor.matmul(
                    psy[:],
                    dt3[:, do * FO_Q + fo, :],
                    hs[:, fo, cs],
                    start=(fo == 0),
                    stop=(fo == FO_Q - 1),
                )
            nc.vector.tensor_mul(out=yo[:, do, :], in0=psy[:], in1=w2b[:, cs])
            if do % 2 == 1:
                oq.dma_start(
                    yt3[:, cc * DO + do - 1 : cc * DO + do + 1, :],
                    yo[:, do - 1 : do + 1, :],
                )


def _build(shapes: tuple):
    """Bass/Tile program: four quarter-expert SwiGLU tasks per core."""
    nc = bacc.Bacc(
        "TRN2",
        target_bir_lowering=False,
        debug=False,
        enable_asserts=False,
        num_devices=N_CORES,
    )
    NT = len(shapes)
    xt, w2, gw, uw, dw, yt = [], [], [], [], [], []
    for t, (C, nch) in enumerate(shapes):
        xt.append(nc.dram_tensor(f"xt{t}", [P, DO * C], BF16, kind="ExternalInput").ap())
        w2.append(nc.dram_tensor(f"w2{t}", [1, C], BF16, kind="ExternalInput").ap())
        gw.append(nc.dram_tensor(f"gw{t}", [P, FO_Q * DO * P], BF16, kind="ExternalInput").ap())
        uw.append(nc.dram_tensor(f"uw{t}", [P, FO_Q * DO * P], BF16, kind="ExternalInput").ap())
        dw.append(nc.dram_tensor(f"dw{t}", [P, DO * FO_Q * P], BF16, kind="ExternalInput").ap())
        yt.append(nc.dram_tensor(f"yt{t}", [P, DO * C], BF16, kind="ExternalOutput").ap())
    g3 = [g.rearrange("p (x f) -> p x f", f=P) for g in gw]
    u3 = [u.rearrange("p (x f) -> p x f", f=P) for u in uw]
    d3 = [d.rearrange("p (x f) -> p x f", f=P) for d in dw]
    xt3 = [x.rearrange("p (x c) -> p x c", c=C // nch) for x, (C, nch) in zip(xt, shapes)]
    yt3 = [y.rearrange("p (x c) -> p x c", c=C // nch) for y, (C, nch) in zip(yt, shapes)]

    with tile.TileContext(nc) as tc:
        with (
            tc.tile_pool(name="big", bufs=1) as big,
            tc.tile_pool(name="sp", bufs=4) as sp,
            tc.tile_pool(name="yp", bufs=2) as yp,
            tc.tile_pool(name="pg", bufs=2, space="PSUM") as pgp,
            tc.tile_pool(name="pu", bufs=2, space="PSUM") as pup,
            tc.tile_pool(name="pd", bufs=3, space="PSUM") as pdp,
            tc.tile_pool(name="pw", bufs=1, space="PSUM") as pwp,
        ):
            # ── persistent tiles ────────────────────────────────────────
            wkt = big.tile([P, P], BF16, tag="wkt")
            xts, w2r, w2b, gts, uts, dts, hss = [], [], [], [], [], [], []
            for t, (C, nch) in enumerate(shapes):
                chunk = C // nch
                xts.append(big.tile([P, nch * DO, chunk], BF16, tag=f"xts{t}", name=f"xts{t}"))
                w2r.append(big.tile([1, C], BF16, tag=f"w2r{t}", name=f"w2r{t}"))
                w2b.append(big.tile([P, C], BF16, tag=f"w2b{t}", name=f"w2b{t}"))
                gts.append(big.tile([P, FO_Q * DO, P], BF16, tag=f"gts{t}", name=f"gts{t}"))
                uts.append(big.tile([P, FO_Q * DO, P], BF16, tag=f"uts{t}", name=f"uts{t}"))
                dts.append(big.tile([P, DO * FO_Q, P], BF16, tag=f"dts{t}", name=f"dts{t}"))
                hss.append(big.tile([P, FO_Q, C], BF16, tag=f"hss{t}", name=f"hss{t}"))

            # ── PE warmup: memset a tile (no DMA dep), run dummy matmuls
            # so the HAM clock gate reaches 2.4 GHz before real work ──────
            nc.vector.memset(wkt[:], 0.25)
            wps = pwp.tile([P, P], F32, tag="wps")

            def _warm(n):
                for _ in range(n):
                    nc.tensor.matmul(wps[:], wkt[:], wkt[:], start=True, stop=True)

            _warm(N_WARM)

            # ── DMA schedule: ONE input queue (gpsimd), strict priority:
            # task-by-task (tokens, gate/up slabs, down, w2 row) ─────────
            for t, (C, nch) in enumerate(shapes):
                chunk = C // nch
                if t == 0:
                    nc.gpsimd.dma_start(xts[t][:, 0:4], xt3[t][:, 0:4])
                    nc.gpsimd.dma_start(xts[t][:, 4:DO], xt3[t][:, 4:DO])
                    wplan = [(0, 1), (1, 2), (2, 4)]
                else:
                    nc.gpsimd.dma_start(xts[t][:, 0:DO], xt3[t][:, 0:DO])
                    wplan = [(0, 2), (2, 4)]
                for lo, hi in wplan:
                    nc.gpsimd.dma_start(gts[t][:, lo * DO : hi * DO], g3[t][:, lo * DO : hi * DO])
                    nc.gpsimd.dma_start(uts[t][:, lo * DO : hi * DO], u3[t][:, lo * DO : hi * DO])
                for cc in range(1, nch):
                    csl = slice(cc * DO, (cc + 1) * DO)
                    nc.gpsimd.dma_start(xts[t][:, csl], xt3[t][:, csl])
                nc.gpsimd.dma_start(dts[t][:], d3[t][:])
                nc.gpsimd.dma_start(w2r[t][:], w2[t])
                nc.gpsimd.partition_broadcast(w2b[t][:], w2r[t][:])

            # ── tasks: heaviest first; P1 then P2 per task ──────────────
            for t, (C, nch) in enumerate(shapes):
                chunk = C // nch
                _task_phase1(
                    nc, gts[t], uts[t], xts[t], hss[t], (sp, pgp, pup),
                    chunk, nch, f"t{t}", warm_fn=_warm if t == 0 else None,
                )
                _task_phase2(
                    nc, dts[t], hss[t], w2b[t], yt3[t], (yp, pdp), chunk, nch,
                    f"t{t}", nc.scalar,
                )
    nc.finalize()
    return nc


def _get_program(key):
    if key not in _BUILD_CACHE:
        _BUILD_CACHE[key] = _build(key)
    return _BUILD_CACHE[key]


def _sigmoid(z):
    return 1.0 / (1.0 + np.exp(-z))


def _route(xf32, macro_w, micro_w):
    """Host routers in float64. Returns group index per token and per-token
    weights for the 2 experts of the selected group (float32)."""
    xf = xf32.astype(np.float64)
    ms = _sigmoid(xf @ macro_w.astype(np.float64))  # [T, G]
    g_sel = np.argmax(ms, axis=1)
    T = xf.shape[0]
    mval = ms[np.arange(T), g_sel]
    mv = mval / (mval + EPS)

    w2 = np.zeros((T, 2), np.float64)
    for g in range(NUM_GROUPS):
        idx = np.nonzero(g_sel == g)[0]
        if idx.size == 0:
            continue
        s = _sigmoid(xf[idx] @ micro_w[g].astype(np.float64))  # [n, 2]
        denom = np.maximum(s[:, 0], s[:, 1]) + np.minimum(s[:, 0], s[:, 1]) + EPS
        w2[idx, 0] = mv[idx] * s[:, 0] / denom
        w2[idx, 1] = mv[idx] * s[:, 1] / denom
    return g_sel, w2.astype(np.float32)


def _pick_capacity(n: int):
    n = max(n, 16)
    nch = max(1, -(-n // 512))
    chunk = -(-n // nch)
    chunk = -(-chunk // 4) * 4
    return chunk * nch, nch


def _pack_w(w):
    """[1024, 512] f32 -> [128, 4096] bf16, layout [p, outer4, inner8, 128]."""
    return np.ascontiguousarray(
        w.reshape(8, P, N_QUART, P).transpose(1, 2, 0, 3).reshape(P, FO_Q * DO * P)
    ).astype(ml_dtypes.bfloat16)


def _pack_wd(w):
    """[512, 1024] f32 -> [128, 4096] bf16, layout [p, outer8, inner4, 128]."""
    return np.ascontiguousarray(
        w.reshape(N_QUART, P, 8, P).transpose(1, 2, 0, 3).reshape(P, DO * FO_Q * P)
    ).astype(ml_dtypes.bfloat16)


def _pack_x(xg, C, nch, chunk):
    """tokens [n, 1024] f32 -> [128, 8*C] bf16, layout [p, cc, do, c]."""
    n = xg.shape[0]
    z = np.zeros((D_MODEL, C), np.float32)
    if n:
        z[:, :n] = xg.T
    return np.ascontiguousarray(
        z.reshape(DO, P, nch, chunk).transpose(1, 2, 0, 3).reshape(P, DO * C)
    ).astype(ml_dtypes.bfloat16)


def _unpack_y(r, C, nch, chunk, n):
    """[128, 8*C] bf16 -> [n, 1024] f32 token-major partial output."""
    y = (
        np.asarray(r, dtype=np.float32)
        .reshape(P, nch, DO, chunk)
        .transpose(2, 0, 1, 3)
        .reshape(D_MODEL, C)
    )
    return y[:, :n].T


def kernel(x, macro_w, micro_w, gate_w, up_w, down_w):
    global LAST_RESULTS
    x = np.asarray(x)
    B, S, D = x.shape
    T = B * S
    xf = np.ascontiguousarray(x.reshape(T, D).astype(np.float32, copy=False))

    g_sel, w2 = _route(xf, np.asarray(macro_w), np.asarray(micro_w))
    idx_by_g = [np.nonzero(g_sel == g)[0] for g in range(NUM_GROUPS)]
    sizes = np.array([ix.size for ix in idx_by_g])

    # tasks: one quarter-expert per group per core, heaviest group first
    order = [int(g) for g in np.argsort(-sizes, kind="stable")]
    shapes = []
    for g in order:
        C, nch = _pick_capacity(int(sizes[g]))
        shapes.append((C, nch))
    shapes = tuple(shapes)
    nc = _get_program(shapes)

    gate_w = np.asarray(gate_w, np.float32)
    up_w = np.asarray(up_w, np.float32)
    down_w = np.asarray(down_w, np.float32)

    in_maps = []
    for c in range(N_CORES):
        j = c % 2  # local expert within each group
        q = c // 2  # F-quarter
        fsl = slice(q * F_Q, (q + 1) * F_Q)
        m = {}
        for t, g_of in enumerate(order):
            C, nch = shapes[t]
            chunk = C // nch
            e = 2 * g_of + j
            ix = idx_by_g[g_of]
            m[f"xt{t}"] = _pack_x(xf[ix], C, nch, chunk)
            w2row = np.zeros((1, C), np.float32)
            w2row[0, : ix.size] = w2[ix, j]
            m[f"w2{t}"] = w2row.astype(ml_dtypes.bfloat16)
            m[f"gw{t}"] = _pack_w(gate_w[e][:, fsl])
            m[f"uw{t}"] = _pack_w(up_w[e][:, fsl])
            m[f"dw{t}"] = _pack_wd(down_w[e][fsl, :])
        in_maps.append(m)

    res = run_bass_kernel_spmd(nc, in_maps, core_ids=list(range(N_CORES)))
    LAST_RESULTS = res

    y = np.zeros((T, D), np.float32)
    for c in range(N_CORES):
        for t, g_of in enumerate(order):
            C, nch = shapes[t]
            chunk = C // nch
            ix = idx_by_g[g_of]
            if ix.size:
                y[ix] += _unpack_y(res.results[c][f"yt{t}"], C, nch, chunk, ix.size)
    return y.reshape(B, S, D)
